# revision 11
# baseline (speedup 1.0000x reference)
"""Back-projection (nn_BackProjectionNet) Trainium2 Bass kernel, v4.

v4: shear moved on-chip. Per (b, q) one contiguous "diagonal band" DMA
(partition w holds c2 rows [smin+w, smin+w+R)) replaces the 322 scatter
DMAs (~87k tiny descriptors); the shear is then done by cheap engine
copies (Act/DVE/Pool round-robin) using the same progression APs, and
the output DMA writes contiguous 6 KB runs per partition.

Full inputs in, full outputs out. Sharding: z (last dim, 192) split over 8
cores, 24 z-planes each; no collectives.

Math (per angle-set): out = E01.T@T + ED1.T@(r*T) + Q.(EQ.T@T), with
  E01[K,x]=(1-k)s, E01[K+1,x]=k*s       (k-lerp folded into one-hot)
  ED1[K,x]=-s, ED1[K+1,x]=+s            (r-term via r-scaled rhs)
  EQ[K,x]=+s, EQ[K+1,x]=-2s, EQ[K+2,x]=+s, weighted by Q=max(r+k-1,0)
exact bilinear identity: G0+p(G1-G0)+max(p-1,0)(G2-2G1+G0), p=r+k.

Device: per (quarter, base): T-hat tile [w_b, (y32,d96) x2] bf16; half0
shear-DMA'd from per-base DRAM c2 (rows duplicated so run lengths 1 and 2
both lower to 3-dim arithmetic-progression APs); half1 = r*half0 (Pool).
PE accumulates E01/ED1 into per-ch PSUM acc, EQ into pd; Q-stream does
m = Q.pd (Pool direct from PSUM, or Act-evac + DVE bf16-2x mul), aw += m
(DVE 2x). Evac per quarter, PE-transpose unwind, 1/120 folded into E.
"""

import math
import numpy as np

import concourse.bacc as bacc
import concourse.mybir as mybir
from concourse import tile
from concourse.ap import AP
from concourse.bass_utils import run_bass_kernel_spmd

NA, LR, LZ, PAD = 120, 128, 192, 27
LP = LR + 2 * PAD          # 182
CEN = (LP - 1) / 2.0       # 90.5
N_CORES = 8
ZC = LZ // N_CORES         # 24
ROWS = 208                 # c2 rows per base (true row -1 .. 206)
WMAX = 96
NBASE = 16
NSLOT = 4
DT = mybir.dt
S_NORM = float(np.float32(1.0 / (120.0 + 1e-11)))

F32, BF16 = DT.float32, DT.bfloat16
CROW = 192                 # c2 elems per row: 2 copies x 96

# knobs
PATH_PATTERN = ['A', 'C', 'D', 'A', 'C']
COPY_PATTERN = ['A', 'V', 'P']   # shear-copy engines: Act, DVE, Pool
RBAND = 32                       # max c2-row span of any (b, q) band


def _job_slots():
    slots = {}
    for b in range(NBASE):
        if b == 0:
            slots[b] = [(60, 0), (90, 30), None, None]
        elif b == 15:
            slots[b] = [(75, 15), None, None, (105, 45)]
        else:
            slots[b] = [((60 + b) % 120, b), (90 - b, 30 - b),
                        (60 - b, 120 - b), (90 + b, 30 + b)]
    return slots


def _geom(b, xm):
    a = 2 * math.pi * b / NA
    cpa, spa = math.sin(a), math.cos(a)
    cpx = -cpa if xm else cpa
    yc = np.arange(PAD, PAD + LR, dtype=np.float64) - CEN
    xc = np.arange(PAD, PAD + LR, dtype=np.float64) - CEN
    ay = spa * yc + CEN
    bx = cpx * xc
    return ay, bx


def host_prep():
    slots = _job_slots()
    sets = []
    for b in range(NBASE):
        sets.append((b, False))
        if b != 0:
            sets.append((b, True))
    nset = len(sets)                      # 31

    # per-base geometry: union u-window over plain/xm
    Sb = {}
    KOFF = {}
    WB = {}
    kf_all = {}
    Kf_all = {}
    r_all = {}
    for b in range(NBASE):
        ay, bx_p = _geom(b, False)
        _, bx_m = _geom(b, True)
        Sf = np.floor(ay).astype(np.int64)
        Kf_p = np.floor(bx_p).astype(np.int64)
        Kf_m = np.floor(bx_m).astype(np.int64)
        koff = int(min(Kf_p.min(), Kf_m.min()))
        wb = int(max(Kf_p.max(), Kf_m.max())) - koff + 3
        assert wb <= WMAX, (b, wb)
        # c2 row index = true_row + 1; S'[y] = Sf + koff + 1
        Sb[b] = (Sf + koff + 1).astype(np.int64)
        KOFF[b] = koff
        WB[b] = wb
        Kf_all[(b, False)] = Kf_p
        Kf_all[(b, True)] = Kf_m
        kf_all[(b, False)] = bx_p - np.floor(bx_p)
        kf_all[(b, True)] = bx_m - np.floor(bx_m)
        r_all[b] = ay - np.floor(ay)
        assert Sb[b].min() >= 0
        assert Sb[b].max() + wb - 1 < ROWS, (b, Sb[b].max() + wb)

    # E matrices [WMAX, 128] per set x3 groups; Q tables
    E = np.zeros((nset, 3, WMAX, 128), np.float64)
    Q = np.zeros((nset, 128, 128), np.float64)      # [x, y]
    for si, (b, xm) in enumerate(sets):
        Ku = (Kf_all[(b, xm)] - KOFF[b]).astype(np.int64)   # [x] in [0, wb-3]
        k = kf_all[(b, xm)]
        r = r_all[b]
        xr = np.arange(128)
        E[si, 0][Ku, xr] = (1.0 - k) * S_NORM
        E[si, 0][Ku + 1, xr] = k * S_NORM
        E[si, 1][Ku, xr] = -S_NORM
        E[si, 1][Ku + 1, xr] = S_NORM
        E[si, 2][Ku, xr] = S_NORM
        E[si, 2][Ku + 1, xr] = -2.0 * S_NORM
        E[si, 2][Ku + 2, xr] = S_NORM
        Q[si] = np.maximum(r[None, :] + k[:, None] - 1.0, 0.0)  # [x, y]

    # r table [16, 128]
    rT = np.zeros((NBASE, 128), np.float64)
    for b in range(NBASE):
        rT[b] = r_all[b]

    # shear progressions per (b, q): list of (L, y0, dy, n, S0, dS)
    progs = {}
    n_dma = 0
    for b in range(NBASE):
        S = Sb[b]
        for q in range(4):
            runs = []
            y = 32 * q
            while y < 32 * (q + 1):
                y1 = y
                while y1 + 1 < 32 * (q + 1) and S[y1 + 1] == S[y]:
                    y1 += 1
                runs.append((y, y1 - y + 1, int(S[y])))
                y = y1 + 1
            pl = []
            for L in (1, 2):
                sub = [(y0, s) for (y0, ln, s) in runs if ln == L]
                i = 0
                while i < len(sub):
                    if i + 1 < len(sub):
                        dy = sub[i + 1][0] - sub[i][0]
                        dS = sub[i + 1][1] - sub[i][1]
                        j = i + 1
                        while (j + 1 < len(sub)
                               and sub[j + 1][0] - sub[j][0] == dy
                               and sub[j + 1][1] - sub[j][1] == dS):
                            j += 1
                        n = j - i + 1
                    else:
                        dy = dS = 0
                        n = 1
                    pl.append((L, sub[i][0], dy, n, sub[i][1], dS))
                    i += n
            progs[(b, q)] = pl
            n_dma += len(pl)

    # per-(b, q) band: contiguous c2-row window [smin, smax] covering S
    band = {}
    for b in range(NBASE):
        S = Sb[b]
        for q in range(4):
            Sq = S[32 * q:32 * (q + 1)]
            smin, smax = int(Sq.min()), int(Sq.max())
            assert smax - smin + 1 <= RBAND, (b, q, smax - smin + 1)
            band[(b, q)] = (smin, smax - smin + 1)

    # Q-path per set: interleave A (act evac + pool mul), C (act evac +
    # dve mul), D (dve mul direct from psum)
    qpath = {}
    ctr = 0
    for si, (b, xm) in enumerate(sets):
        if b == 0:
            qpath[si] = None
            continue
        qpath[si] = PATH_PATTERN[ctr % len(PATH_PATTERN)]
        ctr += 1

    return dict(slots=slots, sets=sets, E=E, Q=Q, rT=rT, progs=progs,
                Sb=Sb, WB=WB, KOFF=KOFF, qpath=qpath, n_dma=n_dma,
                band=band, ident=np.eye(128, dtype=np.float32))


def host_inputs(tabs, image, core):
    """c2 per core: [NBASE, ROWS, 2, 96] bf16 (row = true_row + 1)."""
    import ml_dtypes
    z0 = core * ZC
    img = np.asarray(image)[0, :, :, z0:z0 + ZC]               # [120,128,ZC]
    img_p = np.pad(img, ((0, 0), (PAD, PAD), (0, 0)))          # [120,182,ZC]
    slots = tabs["slots"]
    c2 = np.zeros((NBASE, ROWS, 2, 96), np.float32)
    for b in range(NBASE):
        for s in range(NSLOT):
            j = slots[b][s]
            if j is None:
                continue
            mp, mf = j
            sl = img_p[mp] + img_p[mf][::-1]                   # [182, ZC]
            c2[b, 1:1 + LP, 0, s * ZC:(s + 1) * ZC] = sl
            c2[b, 1:1 + LP, 1, s * ZC:(s + 1) * ZC] = sl
    return c2.astype(ml_dtypes.bfloat16)


def build_nc(tabs, repeat=1, nbases=NBASE, nquarters=4):
    sets = tabs["sets"]
    progs = tabs["progs"]
    WB = tabs["WB"]
    Sb = tabs["Sb"]
    band = tabs["band"]
    qpath = tabs["qpath"]
    nset = len(sets)
    set_idx = {bs: i for i, bs in enumerate(sets)}

    nc = bacc.Bacc("TRN2", target_bir_lowering=False, debug=False,
                   num_devices=N_CORES)
    d_c2 = nc.dram_tensor("c2", [NBASE, ROWS, 2, 96], BF16,
                          kind="ExternalInput")
    d_E = nc.dram_tensor("e_tab", [WMAX, nset * 3 * 128], BF16,
                         kind="ExternalInput")
    d_Q = nc.dram_tensor("q_tab", [128, nset * 128], BF16,
                         kind="ExternalInput")
    d_r = nc.dram_tensor("r_tab", [NBASE * 128], BF16, kind="ExternalInput")
    d_I = nc.dram_tensor("ident", [128, 128], F32, kind="ExternalInput")
    d_out = nc.dram_tensor("out", [2, 128, 128, 12], F32,
                           kind="ExternalOutput")

    c2t = d_c2[:].tensor

    with tile.TileContext(nc) as tc:
        with tc.tile_pool(name="const", bufs=1) as cpool, \
             tc.tile_pool(name="work", bufs=4) as wpool, \
             tc.tile_pool(name="mbufs", bufs=3) as mpool, \
             tc.tile_pool(name="accs", bufs=1) as apool, \
             tc.tile_pool(name="fin", bufs=2) as fpool, \
             tc.tile_pool(name="psum", bufs=1, space="PSUM") as ppool:

            # ---- constants (outside repeat loop) ----
            t_E = cpool.tile([WMAX, nset * 3 * 128], BF16, tag="etab")
            nc.sync.dma_start(out=t_E[:], in_=d_E[:])
            t_Q = cpool.tile([128, nset * 128], BF16, tag="qtab")
            nc.sync.dma_start(out=t_Q[:], in_=d_Q[:])
            t_r = cpool.tile([128, NBASE * 128], BF16, tag="rtab")
            nc.sync.dma_start(
                out=t_r[:],
                in_=AP(d_r[:].tensor, 0, [[0, 128], [1, NBASE * 128]]))
            t_I = cpool.tile([128, 128], F32, tag="ident")
            nc.sync.dma_start(out=t_I[:], in_=d_I[:])

            def E_ap(si, g, wb):          # lhsT [wb, 128] bf16
                return AP(t_E[:].tensor, (si * 3 + g) * 128,
                          [[nset * 3 * 128, wb], [1, 128]])

            def Q_ap(si, q):              # [128x, (d48 bcast), (y32)] bf16
                return AP(t_Q[:].tensor, si * 128 + 32 * q,
                          [[nset * 128, 128], [0, 48], [1, 32]])

            def r_ap(b, q, wb):           # [wb, (y32), (d96 bcast)] bf16
                return AP(t_r[:].tensor, b * 128 + 32 * q,
                          [[NBASE * 128, wb], [1, 32], [0, 96]])

            def body():
                out_t = apool.tile([128, 128 * 48], F32, tag="outbuf")
                for q in range(nquarters):
                    accs = [ppool.tile([128, 512], F32, tag=f"acc{c}",
                                       name=f"acc{c}")
                            for c in range(3)]
                    aw = apool.tile([128, 1536], BF16, tag="aw")
                    n_accmm = sum((1 if b == 0 else 2) * (1 if b == 0 else 2)
                                  for b in range(nbases))
                    # per ch: one E01 mm per set + one ED1 mm per set (b>0)
                    n_accmm = 0
                    for b in range(nbases):
                        ns = 1 if b == 0 else 2
                        n_accmm += ns * (1 if b == 0 else 2)
                    mm_done = [0] * 3
                    first_q_set = True
                    for b in range(nbases):
                        wb = WB[b]
                        # ---- diagonal band load (one big DMA) ----
                        smin, rq = band[(b, q)]
                        bd = wpool.tile([WMAX, RBAND * CROW], BF16,
                                        tag="band")
                        bdt = bd[:].tensor
                        bdo = bd[:].offset
                        nc.sync.dma_start(
                            out=AP(bdt, bdo,
                                   [[RBAND * CROW, wb], [1, rq * CROW]]),
                            in_=AP(c2t, (b * ROWS + smin) * CROW,
                                   [[CROW, wb], [1, rq * CROW]]))
                        th = wpool.tile([WMAX, 2 * 3072], BF16, tag="tshear")
                        tt = th[:].tensor
                        toff = th[:].offset
                        # ---- on-chip shear (coalesced progressions) ----
                        for pi, (L, y0, dy, n, S0, dS) in enumerate(progs[(b, q)]):
                            nel = 96 * L
                            dst = AP(tt, toff + (y0 - 32 * q) * 96,
                                     [[2 * 3072, wb], [96 * dy, n],
                                      [1, nel]])
                            src = AP(bdt, bdo + (S0 - smin) * CROW,
                                     [[RBAND * CROW, wb], [CROW * dS, n],
                                      [1, nel]])
                            ce = COPY_PATTERN[pi % len(COPY_PATTERN)]
                            if ce == 'A':
                                nc.scalar.copy(dst, src)
                            elif ce == 'V':
                                nc.vector.tensor_copy(dst, src)
                            else:
                                nc.gpsimd.tensor_copy(dst, src)
                        # ---- T_r = r * T (Pool) ----
                        if b != 0:
                            nc.gpsimd.tensor_mul(
                                AP(tt, toff + 3072,
                                   [[2 * 3072, wb], [96, 32], [1, 96]]),
                                AP(tt, toff,
                                   [[2 * 3072, wb], [96, 32], [1, 96]]),
                                r_ap(b, q, wb))
                        b_sets = [(b, False)] + ([(b, True)] if b else [])
                        for (bb, xm) in b_sets:
                            si = set_idx[(bb, xm)]
                            doff = 48 if xm else 0

                            def rhs(ch, half):
                                return AP(tt, toff + half * 3072
                                          + doff + ch * 16,
                                          [[2 * 3072, wb], [1, 16], [96, 32]])

                            # E01 -> acc
                            for ch in range(3):
                                mm_done[ch] += 1
                                nc.tensor.matmul(
                                    accs[ch][:], E_ap(si, 0, wb),
                                    rhs(ch, 0),
                                    start=(mm_done[ch] == 1),
                                    stop=(mm_done[ch] == n_accmm))
                            # ED1 -> acc (uses T_r)
                            if b != 0:
                                for ch in range(3):
                                    mm_done[ch] += 1
                                    nc.tensor.matmul(
                                        accs[ch][:], E_ap(si, 1, wb),
                                        rhs(ch, 1),
                                        start=False,
                                        stop=(mm_done[ch] == n_accmm))
                            if b == 0:
                                continue
                            # EQ -> pd ; Q-stream
                            pd = ppool.tile([128, 1536], F32, tag="pd")
                            for ch in range(3):
                                nc.tensor.matmul(
                                    pd[:, 512 * ch:512 * (ch + 1)],
                                    E_ap(si, 2, wb), rhs(ch, 0),
                                    start=True, stop=True)
                            pd3 = AP(pd[:].tensor, pd[:].offset,
                                     [[1536, 128], [32, 48], [1, 32]])
                            aw3 = AP(aw[:].tensor, aw[:].offset,
                                     [[1536, 128], [32, 48], [1, 32]])
                            # GPSIMD cannot read PSUM on HW: all paths
                            # evacuate pd via Act first, except path D
                            # (DVE mul direct from PSUM).
                            path = qpath[si]
                            if first_q_set:
                                dst3 = aw3
                            else:
                                m = mpool.tile([128, 1536], BF16,
                                               tag="mbuf")
                                m3 = AP(m[:].tensor, m[:].offset,
                                        [[1536, 128], [32, 48], [1, 32]])
                                dst3 = m3
                            if path == 'D':
                                nc.vector.tensor_mul(dst3, pd3, Q_ap(si, q))
                            else:
                                psb = mpool.tile([128, 1536], BF16,
                                                 tag="psb")
                                nc.scalar.copy(psb[:], pd[:])
                                psb3 = AP(psb[:].tensor, psb[:].offset,
                                          [[1536, 128], [32, 48], [1, 32]])
                                if path == 'A':
                                    nc.gpsimd.tensor_mul(dst3, psb3,
                                                         Q_ap(si, q))
                                else:
                                    nc.vector.tensor_mul(dst3, psb3,
                                                         Q_ap(si, q))
                            if not first_q_set:
                                nc.vector.tensor_add(aw[:], aw[:], m[:])
                            first_q_set = False
                    # ---- evac quarter ----
                    for ch in range(3):
                        nc.vector.tensor_add(
                            AP(out_t[:].tensor, 32 * q * 48 + ch * 16,
                               [[128 * 48, 128], [1, 16], [48, 32]]),
                            AP(accs[ch][:].tensor, accs[ch][:].offset,
                               [[512, 128], [32, 16], [1, 32]]),
                            AP(aw[:].tensor, aw[:].offset + ch * 512,
                               [[1536, 128], [32, 16], [1, 32]]))

                # ---- unwind: out = A + B^T (scale folded into E) ----
                for zc2 in range(2):
                    bt = ppool.tile([128, 1536], F32, tag="pd")
                    for zl in range(12):
                        z = zc2 * 12 + zl
                        nc.tensor.transpose(
                            bt[:, 128 * zl:128 * (zl + 1)],
                            AP(out_t[:].tensor, 24 + z,
                               [[128 * 48, 128], [48, 128]]),
                            t_I[:])
                    t_fin = fpool.tile([128, 128 * 12], F32, tag="fin")
                    nc.vector.tensor_add(
                        t_fin[:],
                        AP(out_t[:].tensor, zc2 * 12,
                           [[128 * 48, 128], [48, 128], [1, 12]]),
                        AP(bt[:].tensor, bt[:].offset,
                           [[1536, 128], [1, 128], [128, 12]]))
                    nc.sync.dma_start(
                        out=AP(d_out[:].tensor, zc2 * (128 * 128 * 12),
                               [[128 * 12, 128], [1, 128 * 12]]),
                        in_=t_fin[:])

            if repeat == 1:
                body()
            else:
                with tc.For_i(0, repeat, 1):
                    body()

    nc.compile()
    return nc


# ---------------------------------------------------------------- entry

_CACHE = {}


def _get(repeat=1):
    key = ("k8", repeat)
    if key not in _CACHE:
        tabs = host_prep()
        nc = build_nc(tabs, repeat=repeat)
        _CACHE[key] = (tabs, nc)
    return _CACHE[key]


def make_in_maps(tabs, image):
    import ml_dtypes
    nset = len(tabs["sets"])
    e_bf = np.ascontiguousarray(
        np.transpose(tabs["E"], (2, 0, 1, 3)).reshape(WMAX, nset * 3 * 128)
    ).astype(ml_dtypes.bfloat16)
    q_bf = np.ascontiguousarray(
        np.transpose(tabs["Q"], (1, 0, 2)).reshape(128, nset * 128)
    ).astype(ml_dtypes.bfloat16)
    r_bf = tabs["rT"].reshape(-1).astype(ml_dtypes.bfloat16)
    in_maps = []
    for c in range(N_CORES):
        m = {"c2": host_inputs(tabs, image, c),
             "e_tab": e_bf, "q_tab": q_bf, "r_tab": r_bf,
             "ident": tabs["ident"]}
        in_maps.append(m)
    return in_maps


def run_built(tabs, nc, image):
    in_maps = make_in_maps(tabs, image)
    res = run_bass_kernel_spmd(nc, in_maps, list(range(N_CORES)), trace=False)
    outs = []
    for c in range(N_CORES):
        o = res.results[c]["out"]                 # [2, x, y, 12]
        o = np.concatenate([o[0], o[1]], axis=2)  # [x, y, ZC]
        outs.append(np.transpose(o, (1, 0, 2)))   # [y, x, ZC]
    full = np.concatenate(outs, axis=2)           # [128, 128, 192]
    return full[None].astype(np.float32)


def kernel(image):
    image = np.asarray(image, dtype=np.float32)
    tabs, nc = _get(repeat=1)
    return run_built(tabs, nc, image)

